# revision 32
# baseline (speedup 1.0000x reference)
"""Conditional linear (MoE routing) Trainium2 kernel.

Problem: x [32, 1024, 1024] f32, task_id [32] int, W [16, 1024*1024] f32.
  out[b] = x[b] @ W[task_id[b]].reshape(FOUT, FIN).T          # [B, N, FOUT]
Returns (out, task_id), matching the reference.

Sharding: data-parallel over batch — each of the 8 NeuronCores gets 4
batches. The per-sample weight gather W[task_id[b]] happens on the host
while building the per-core input maps (it is pure data movement); each
core then runs 4 independent 1024x1024x1024 matmuls on TensorE in
float32r (fp32 bits, FP22 multiply, fp32 accumulate) at full PE rate.

Both matmul operands need the contraction dim (fin) on SBUF partitions,
so the host feeds pre-transposed views: xT [fin, n] and wT [fin, fout].
"""

import numpy as np

B, N_SEQ, FIN, FOUT, N_TASKS = 32, 1024, 1024, 1024, 16
N_CORES = 8
B_LOC = B // N_CORES  # 4 batches per core

KP = 128              # contraction tile (partition dim)
KT = FIN // KP        # 8 k-tiles
MT = N_SEQ // 128     # 8 m-tiles (output partition dim)
NF = 512              # moving free dim per matmul (one PSUM bank of fp32)
NT = FOUT // NF       # n-tiles
MH = MT // 2          # m-tiles per output staging half

_NC_CACHE = None


def _patch_tile_drain():
    """This walrus build caps sync waits at 1 per instruction (2 for event
    sems), but TileContext's tail drain piles every outstanding proc's sem
    wait onto a single Drain.  Spread them over single-wait SP nops."""
    import concourse.mybir as mybir
    from concourse.tile import TileContext
    from bass_rust import ScopedClock

    if getattr(TileContext, "_drain_patched", False):
        return

    def _drain_and_barrier(self, tick_clock, wait_clock):
        nc = self.nc
        probe = nc.sync.nop(nofuse=True)
        wait_clock.add_sem_waits(
            probe.ins, ScopedClock({None: tick_clock.global_clock})
        )
        si = probe.ins.sync_info
        waits = list(si.on_wait) if si is not None and si.on_wait else []
        if len(waits) > 1:
            probe.ins.sync_info = mybir.SyncInfo(on_wait=[waits[0]], on_update=[])
            for w in waits[1:]:
                n = nc.sync.nop(nofuse=True)
                n.ins.sync_info = mybir.SyncInfo(on_wait=[w], on_update=[])
        nc.sync.drain()
        nc.all_engine_barrier()
        assert self.sems is not None
        popped = nc._tile_sem_poison_stack.pop()
        assert popped is self._sem_poison
        # Like nc.clear_and_free_semaphores, minus gpsimd.dma_reset — the
        # DMA-quiesce pseudo that NRT expands into a ~8us token-ring
        # barrier across every engine.  All DMAs are provably complete
        # here (the drain above waited on every DMA completion sem), so
        # resetting sem values alone is enough for re-execution.
        from concourse.bass import compact_to_ranges

        sems = list(self.sems.allocated().values())
        if sems:
            sem_nums = [s.num for s in sems]
            for sem_range in compact_to_ranges(sem_nums):
                nc.gpsimd.sem_clear(sem_range)
        nc.all_engine_barrier()

    TileContext._drain_and_barrier = _drain_and_barrier
    TileContext._drain_patched = True


def _split_excess_waits(nc):
    """Walrus in this build accepts 1 sync wait per instruction (2 on event
    sems).  Tile occasionally emits more (e.g. a self-loading f32r matmul
    whose two operands arrive via two DMAs).  Hoist the excess waits onto
    same-engine NoOps inserted directly before the instruction."""
    import concourse.mybir as mybir

    n_split = 0
    for f in nc.m.functions:
        for bb in f.blocks:
            new_insts = []
            changed = False
            for inst in bb.instructions:
                si = inst.sync_info
                waits = list(si.on_wait) if si is not None and si.on_wait else []
                cap = 2 if type(inst).__name__ == "InstEventSemaphoreOp" else 1
                if len(waits) > cap:
                    changed = True
                    for w in waits[cap:]:
                        nop = mybir.InstNoOp(
                            name=f"{inst.name}-wsplit{n_split}", ins=[], outs=[]
                        )
                        n_split += 1
                        nop.engine = inst.engine
                        nop.sync_info = mybir.SyncInfo(on_wait=[w], on_update=[])
                        new_insts.append(nop)
                    inst.sync_info = mybir.SyncInfo(
                        on_wait=waits[:cap],
                        on_update=list(si.on_update) if si.on_update else [],
                    )
                new_insts.append(inst)
            if changed:
                bb.instructions = new_insts


def _build_nc():
    import concourse.bass as bass
    import concourse.mybir as mybir
    from concourse.tile import TileContext

    _patch_tile_drain()

    nc = bass.Bass()
    f32 = mybir.dt.float32
    f16 = mybir.dt.float16

    # fp16 operands: full-rate PE (FP22 multiply, fp32 accumulate), FWL
    # fast weight loads, and half the input DMA traffic of fp32.  10
    # mantissa bits keep the result within ~5e-4 of the fp32 reference.
    xT = nc.dram_tensor("xT", [B_LOC, FIN, N_SEQ], f16, kind="ExternalInput")
    wT = nc.dram_tensor("wT", [B_LOC, FIN, FOUT], f16, kind="ExternalInput")
    # fp16 output too: halves the output traffic + store tail; the host
    # widens back to fp32.  Adds ~2^-11 relative rounding, still ~5e-4.
    out = nc.dram_tensor("out", [B_LOC, N_SEQ, FOUT], f16, kind="ExternalOutput")

    with TileContext(nc) as tc:
        with (
            tc.tile_pool(name="xp", bufs=3) as xpool,
            tc.tile_pool(name="wp", bufs=3) as wpool,
            tc.tile_pool(name="op", bufs=3) as opool,
            tc.tile_pool(name="ps", bufs=6, space="PSUM") as pspool,
        ):

            for b in range(B_LOC):
                # Per-k-chunk input DMAs so the first matmul of each batch
                # only waits on chunk 0; x rides the SP HWDGE ring, w the
                # ACT ring — they load in parallel, never queued behind
                # each other.
                xt = xpool.tile([128, KT, N_SEQ], f16, name=f"x_{b}", tag="x")
                wt = wpool.tile([128, KT, FOUT], f16, name=f"w_{b}", tag="w")
                for k in range(KT):
                    nc.sync.dma_start(
                        out=xt[:, k, :], in_=xT[b, k * KP:(k + 1) * KP, :]
                    )
                    nc.scalar.dma_start(
                        out=wt[:, k, :], in_=wT[b, k * KP:(k + 1) * KP, :]
                    )
                for quarter in range(4):
                    MQ = MT // 4
                    stage = opool.tile([128, MQ, FOUT], f16, name=f"s_{b}_{quarter}", tag="s")
                    for mi in range(MQ):
                        m = quarter * MQ + mi
                        for j in range(NT):
                            ps = pspool.tile([128, NF], f32, name=f"ps_{b}_{m}_{j}", tag="ps")
                            for k in range(KT):
                                nc.tensor.matmul(
                                    ps,
                                    lhsT=xt[:, k, m * 128:(m + 1) * 128],
                                    rhs=wt[:, k, j * NF:(j + 1) * NF],
                                    start=(k == 0),
                                    stop=(k == KT - 1),
                                )
                            nc.vector.tensor_copy(
                                stage[:, mi, j * NF:(j + 1) * NF], ps
                            )
                    # Output DMAs ride the gpsimd SWDGE path so they never
                    # head-of-line block input prefetch; the final ones take
                    # the by-then-idle sync HWDGE ring (lower latency tail).
                    ring = (
                        nc.sync if (b == B_LOC - 1 and quarter >= 2)
                        else nc.gpsimd
                    )
                    ring.dma_start(
                        out=out[b, quarter * MQ * 128:(quarter + 1) * MQ * 128, :]
                        .rearrange("(m p) f -> p m f", p=128),
                        in_=stage,
                    )
    _split_excess_waits(nc)
    return nc


def _get_nc():
    global _NC_CACHE
    if _NC_CACHE is None:
        _NC_CACHE = _build_nc()
    return _NC_CACHE


def kernel(x, task_id, W):
    from concourse.bass_utils import run_bass_kernel_spmd

    x = np.asarray(x)
    task_id = np.asarray(task_id)
    W = np.asarray(W)

    # Host-side shard prep: fp16-cast + transpose x per batch, gather +
    # transpose the per-task weight (once per distinct task).
    xT = np.ascontiguousarray(x.astype(np.float16).transpose(0, 2, 1))  # [B, FIN, N]
    wT_by_task = {}
    for t in np.unique(task_id):
        t = int(t)
        wT_by_task[t] = np.ascontiguousarray(
            W[t].reshape(FOUT, FIN).astype(np.float16).T
        )

    in_maps = []
    for c in range(N_CORES):
        lo = c * B_LOC
        in_maps.append({
            "xT": xT[lo:lo + B_LOC],
            "wT": np.stack([wT_by_task[int(task_id[b])] for b in range(lo, lo + B_LOC)]),
        })

    nc = _get_nc()
    res = run_bass_kernel_spmd(nc, in_maps, core_ids=list(range(N_CORES)))
    out = np.concatenate(
        [res.results[c]["out"] for c in range(N_CORES)], axis=0
    ).astype(np.float32)
    return (out, task_id)


# revision 34
# speedup vs baseline: 1.0383x; 1.0383x over previous
"""Conditional linear (MoE routing) Trainium2 kernel.

Problem: x [32, 1024, 1024] f32, task_id [32] int, W [16, 1024*1024] f32.
  out[b] = x[b] @ W[task_id[b]].reshape(FOUT, FIN).T          # [B, N, FOUT]
Returns (out, task_id), matching the reference.

Sharding: data-parallel over batch — each of the 8 NeuronCores gets 4
batches. The per-sample weight gather W[task_id[b]] happens on the host
while building the per-core input maps (it is pure data movement); each
core then runs 4 independent 1024x1024x1024 matmuls on TensorE in
float32r (fp32 bits, FP22 multiply, fp32 accumulate) at full PE rate.

Both matmul operands need the contraction dim (fin) on SBUF partitions,
so the host feeds pre-transposed views: xT [fin, n] and wT [fin, fout].
"""

import numpy as np

B, N_SEQ, FIN, FOUT, N_TASKS = 32, 1024, 1024, 1024, 16
N_CORES = 8
B_LOC = B // N_CORES  # 4 batches per core

KP = 128              # contraction tile (partition dim)
KT = FIN // KP        # 8 k-tiles
MT = N_SEQ // 128     # 8 m-tiles (output partition dim)
NF = 512              # moving free dim per matmul (one PSUM bank of fp32)
NT = FOUT // NF       # n-tiles
MH = MT // 2          # m-tiles per output staging half

_NC_CACHE = None


def _patch_tile_drain():
    """This walrus build caps sync waits at 1 per instruction (2 for event
    sems), but TileContext's tail drain piles every outstanding proc's sem
    wait onto a single Drain.  Spread them over single-wait SP nops."""
    import concourse.mybir as mybir
    from concourse.tile import TileContext
    from bass_rust import ScopedClock

    if getattr(TileContext, "_drain_patched", False):
        return

    def _drain_and_barrier(self, tick_clock, wait_clock):
        nc = self.nc
        probe = nc.sync.nop(nofuse=True)
        wait_clock.add_sem_waits(
            probe.ins, ScopedClock({None: tick_clock.global_clock})
        )
        si = probe.ins.sync_info
        waits = list(si.on_wait) if si is not None and si.on_wait else []
        if len(waits) > 1:
            probe.ins.sync_info = mybir.SyncInfo(on_wait=[waits[0]], on_update=[])
            for w in waits[1:]:
                n = nc.sync.nop(nofuse=True)
                n.ins.sync_info = mybir.SyncInfo(on_wait=[w], on_update=[])
        nc.sync.drain()
        nc.all_engine_barrier()
        assert self.sems is not None
        popped = nc._tile_sem_poison_stack.pop()
        assert popped is self._sem_poison
        # Like nc.clear_and_free_semaphores, minus gpsimd.dma_reset — the
        # DMA-quiesce pseudo that NRT expands into a ~8us token-ring
        # barrier across every engine.  All DMAs are provably complete
        # here (the drain above waited on every DMA completion sem), so
        # resetting sem values alone is enough for re-execution.
        from concourse.bass import compact_to_ranges

        sems = list(self.sems.allocated().values())
        if sems:
            sem_nums = [s.num for s in sems]
            for sem_range in compact_to_ranges(sem_nums):
                nc.gpsimd.sem_clear(sem_range)
        nc.all_engine_barrier()

    TileContext._drain_and_barrier = _drain_and_barrier
    TileContext._drain_patched = True


def _split_excess_waits(nc):
    """Walrus in this build accepts 1 sync wait per instruction (2 on event
    sems).  Tile occasionally emits more (e.g. a self-loading f32r matmul
    whose two operands arrive via two DMAs).  Hoist the excess waits onto
    same-engine NoOps inserted directly before the instruction."""
    import concourse.mybir as mybir

    n_split = 0
    for f in nc.m.functions:
        for bb in f.blocks:
            new_insts = []
            changed = False
            for inst in bb.instructions:
                si = inst.sync_info
                waits = list(si.on_wait) if si is not None and si.on_wait else []
                cap = 2 if type(inst).__name__ == "InstEventSemaphoreOp" else 1
                if len(waits) > cap:
                    changed = True
                    for w in waits[cap:]:
                        nop = mybir.InstNoOp(
                            name=f"{inst.name}-wsplit{n_split}", ins=[], outs=[]
                        )
                        n_split += 1
                        nop.engine = inst.engine
                        nop.sync_info = mybir.SyncInfo(on_wait=[w], on_update=[])
                        new_insts.append(nop)
                    inst.sync_info = mybir.SyncInfo(
                        on_wait=waits[:cap],
                        on_update=list(si.on_update) if si.on_update else [],
                    )
                new_insts.append(inst)
            if changed:
                bb.instructions = new_insts


def _build_nc():
    import concourse.bass as bass
    import concourse.mybir as mybir
    from concourse.tile import TileContext

    _patch_tile_drain()

    nc = bass.Bass()
    f32 = mybir.dt.float32
    f16 = mybir.dt.float16

    # fp16 operands: full-rate PE (FP22 multiply, fp32 accumulate), FWL
    # fast weight loads, and half the input DMA traffic of fp32.  10
    # mantissa bits keep the result within ~5e-4 of the fp32 reference.
    xT = nc.dram_tensor("xT", [B_LOC, FIN, N_SEQ], f16, kind="ExternalInput")
    wT = nc.dram_tensor("wT", [B_LOC, FIN, FOUT], f16, kind="ExternalInput")
    # fp16 output too: halves the output traffic + store tail; the host
    # widens back to fp32.  Adds ~2^-11 relative rounding, still ~5e-4.
    out = nc.dram_tensor("out", [B_LOC, N_SEQ, FOUT], f16, kind="ExternalOutput")

    with TileContext(nc) as tc:
        with (
            tc.tile_pool(name="xp", bufs=3) as xpool,
            tc.tile_pool(name="wp", bufs=3) as wpool,
            tc.tile_pool(name="op", bufs=3) as opool,
            tc.tile_pool(name="ps", bufs=8, space="PSUM") as pspool,
        ):

            for b in range(B_LOC):
                # Per-k-chunk input DMAs so the first matmul of each batch
                # only waits on chunk 0; x rides the SP HWDGE ring, w the
                # ACT ring — they load in parallel, never queued behind
                # each other.
                xt = xpool.tile([128, KT, N_SEQ], f16, name=f"x_{b}", tag="x")
                wt = wpool.tile([128, KT, FOUT], f16, name=f"w_{b}", tag="w")
                for k in range(KT):
                    nc.sync.dma_start(
                        out=xt[:, k, :], in_=xT[b, k * KP:(k + 1) * KP, :]
                    )
                    nc.scalar.dma_start(
                        out=wt[:, k, :], in_=wT[b, k * KP:(k + 1) * KP, :]
                    )
                if b == 0:
                    # Batch 0 is gated by the input DMA stream: a k-inner
                    # group needs all 8 chunks within 1.7us while they
                    # arrive over ~10us.  Run the first half k-OUTER over 8
                    # concurrent PSUM banks so each chunk is consumed fully
                    # the moment it lands — no stream stalls.
                    MQ = MT // 4
                    psg = {}
                    for m in range(4):
                        for j in range(NT):
                            psg[m, j] = pspool.tile(
                                [128, NF], f32, name=f"ps_0h_{m}_{j}", tag="ps"
                            )
                    for k in range(KT):
                        for m in range(4):
                            for j in range(NT):
                                nc.tensor.matmul(
                                    psg[m, j],
                                    lhsT=xt[:, k, m * 128:(m + 1) * 128],
                                    rhs=wt[:, k, j * NF:(j + 1) * NF],
                                    start=(k == 0),
                                    stop=(k == KT - 1),
                                    skip_group_check=True,
                                )
                    for quarter in range(2):
                        stage = opool.tile(
                            [128, MQ, FOUT], f16, name=f"s_0_{quarter}", tag="s"
                        )
                        for mi in range(MQ):
                            m = quarter * MQ + mi
                            for j in range(NT):
                                nc.vector.tensor_copy(
                                    stage[:, mi, j * NF:(j + 1) * NF], psg[m, j]
                                )
                        nc.gpsimd.dma_start(
                            out=out[b, quarter * MQ * 128:(quarter + 1) * MQ * 128, :]
                            .rearrange("(m p) f -> p m f", p=128),
                            in_=stage,
                        )
                    quarters = range(2, 4)
                else:
                    quarters = range(4)
                for quarter in quarters:
                    MQ = MT // 4
                    stage = opool.tile([128, MQ, FOUT], f16, name=f"s_{b}_{quarter}", tag="s")
                    for mi in range(MQ):
                        m = quarter * MQ + mi
                        for j in range(NT):
                            ps = pspool.tile([128, NF], f32, name=f"ps_{b}_{m}_{j}", tag="ps")
                            for k in range(KT):
                                nc.tensor.matmul(
                                    ps,
                                    lhsT=xt[:, k, m * 128:(m + 1) * 128],
                                    rhs=wt[:, k, j * NF:(j + 1) * NF],
                                    start=(k == 0),
                                    stop=(k == KT - 1),
                                )
                            nc.vector.tensor_copy(
                                stage[:, mi, j * NF:(j + 1) * NF], ps
                            )
                    # Output DMAs ride the gpsimd SWDGE path so they never
                    # head-of-line block input prefetch; the final ones take
                    # the by-then-idle sync HWDGE ring (lower latency tail).
                    ring = (
                        nc.sync if (b == B_LOC - 1 and quarter >= 2)
                        else nc.gpsimd
                    )
                    ring.dma_start(
                        out=out[b, quarter * MQ * 128:(quarter + 1) * MQ * 128, :]
                        .rearrange("(m p) f -> p m f", p=128),
                        in_=stage,
                    )
    _split_excess_waits(nc)
    return nc


def _get_nc():
    global _NC_CACHE
    if _NC_CACHE is None:
        _NC_CACHE = _build_nc()
    return _NC_CACHE


def kernel(x, task_id, W):
    from concourse.bass_utils import run_bass_kernel_spmd

    x = np.asarray(x)
    task_id = np.asarray(task_id)
    W = np.asarray(W)

    # Host-side shard prep: fp16-cast + transpose x per batch, gather +
    # transpose the per-task weight (once per distinct task).
    xT = np.ascontiguousarray(x.astype(np.float16).transpose(0, 2, 1))  # [B, FIN, N]
    wT_by_task = {}
    for t in np.unique(task_id):
        t = int(t)
        wT_by_task[t] = np.ascontiguousarray(
            W[t].reshape(FOUT, FIN).astype(np.float16).T
        )

    in_maps = []
    for c in range(N_CORES):
        lo = c * B_LOC
        in_maps.append({
            "xT": xT[lo:lo + B_LOC],
            "wT": np.stack([wT_by_task[int(task_id[b])] for b in range(lo, lo + B_LOC)]),
        })

    nc = _get_nc()
    res = run_bass_kernel_spmd(nc, in_maps, core_ids=list(range(N_CORES)))
    out = np.concatenate(
        [res.results[c]["out"] for c in range(N_CORES)], axis=0
    ).astype(np.float32)
    return (out, task_id)


# revision 35
# speedup vs baseline: 1.0421x; 1.0037x over previous
"""Conditional linear (MoE routing) Trainium2 kernel.

Problem: x [32, 1024, 1024] f32, task_id [32] int, W [16, 1024*1024] f32.
  out[b] = x[b] @ W[task_id[b]].reshape(FOUT, FIN).T          # [B, N, FOUT]
Returns (out, task_id), matching the reference.

Sharding: data-parallel over batch — each of the 8 NeuronCores gets 4
batches. The per-sample weight gather W[task_id[b]] happens on the host
while building the per-core input maps (it is pure data movement); each
core then runs 4 independent 1024x1024x1024 matmuls on TensorE in
float32r (fp32 bits, FP22 multiply, fp32 accumulate) at full PE rate.

Both matmul operands need the contraction dim (fin) on SBUF partitions,
so the host feeds pre-transposed views: xT [fin, n] and wT [fin, fout].
"""

import numpy as np

B, N_SEQ, FIN, FOUT, N_TASKS = 32, 1024, 1024, 1024, 16
N_CORES = 8
B_LOC = B // N_CORES  # 4 batches per core

KP = 128              # contraction tile (partition dim)
KT = FIN // KP        # 8 k-tiles
MT = N_SEQ // 128     # 8 m-tiles (output partition dim)
NF = 512              # moving free dim per matmul (one PSUM bank of fp32)
NT = FOUT // NF       # n-tiles
MH = MT // 2          # m-tiles per output staging half

_NC_CACHE = None


def _patch_tile_drain():
    """This walrus build caps sync waits at 1 per instruction (2 for event
    sems), but TileContext's tail drain piles every outstanding proc's sem
    wait onto a single Drain.  Spread them over single-wait SP nops."""
    import concourse.mybir as mybir
    from concourse.tile import TileContext
    from bass_rust import ScopedClock

    if getattr(TileContext, "_drain_patched", False):
        return

    def _drain_and_barrier(self, tick_clock, wait_clock):
        nc = self.nc
        probe = nc.sync.nop(nofuse=True)
        wait_clock.add_sem_waits(
            probe.ins, ScopedClock({None: tick_clock.global_clock})
        )
        si = probe.ins.sync_info
        waits = list(si.on_wait) if si is not None and si.on_wait else []
        if len(waits) > 1:
            probe.ins.sync_info = mybir.SyncInfo(on_wait=[waits[0]], on_update=[])
            for w in waits[1:]:
                n = nc.sync.nop(nofuse=True)
                n.ins.sync_info = mybir.SyncInfo(on_wait=[w], on_update=[])
        nc.sync.drain()
        nc.all_engine_barrier()
        assert self.sems is not None
        popped = nc._tile_sem_poison_stack.pop()
        assert popped is self._sem_poison
        # Like nc.clear_and_free_semaphores, minus gpsimd.dma_reset — the
        # DMA-quiesce pseudo that NRT expands into a ~8us token-ring
        # barrier across every engine.  All DMAs are provably complete
        # here (the drain above waited on every DMA completion sem), so
        # resetting sem values alone is enough for re-execution.
        from concourse.bass import compact_to_ranges

        sems = list(self.sems.allocated().values())
        if sems:
            sem_nums = [s.num for s in sems]
            for sem_range in compact_to_ranges(sem_nums):
                nc.gpsimd.sem_clear(sem_range)
        nc.all_engine_barrier()

    TileContext._drain_and_barrier = _drain_and_barrier
    TileContext._drain_patched = True


def _split_excess_waits(nc):
    """Walrus in this build accepts 1 sync wait per instruction (2 on event
    sems).  Tile occasionally emits more (e.g. a self-loading f32r matmul
    whose two operands arrive via two DMAs).  Hoist the excess waits onto
    same-engine NoOps inserted directly before the instruction."""
    import concourse.mybir as mybir

    n_split = 0
    for f in nc.m.functions:
        for bb in f.blocks:
            new_insts = []
            changed = False
            for inst in bb.instructions:
                si = inst.sync_info
                waits = list(si.on_wait) if si is not None and si.on_wait else []
                cap = 2 if type(inst).__name__ == "InstEventSemaphoreOp" else 1
                if len(waits) > cap:
                    changed = True
                    for w in waits[cap:]:
                        nop = mybir.InstNoOp(
                            name=f"{inst.name}-wsplit{n_split}", ins=[], outs=[]
                        )
                        n_split += 1
                        nop.engine = inst.engine
                        nop.sync_info = mybir.SyncInfo(on_wait=[w], on_update=[])
                        new_insts.append(nop)
                    inst.sync_info = mybir.SyncInfo(
                        on_wait=waits[:cap],
                        on_update=list(si.on_update) if si.on_update else [],
                    )
                new_insts.append(inst)
            if changed:
                bb.instructions = new_insts


def _build_nc():
    import concourse.bass as bass
    import concourse.mybir as mybir
    from concourse.tile import TileContext

    _patch_tile_drain()

    nc = bass.Bass()
    f32 = mybir.dt.float32
    f16 = mybir.dt.float16

    # fp16 operands: full-rate PE (FP22 multiply, fp32 accumulate), FWL
    # fast weight loads, and half the input DMA traffic of fp32.  10
    # mantissa bits keep the result within ~5e-4 of the fp32 reference.
    xT = nc.dram_tensor("xT", [B_LOC, FIN, N_SEQ], f16, kind="ExternalInput")
    wT = nc.dram_tensor("wT", [B_LOC, FIN, FOUT], f16, kind="ExternalInput")
    # fp16 output too: halves the output traffic + store tail; the host
    # widens back to fp32.  Adds ~2^-11 relative rounding, still ~5e-4.
    out = nc.dram_tensor("out", [B_LOC, N_SEQ, FOUT], f16, kind="ExternalOutput")

    with TileContext(nc) as tc:
        with (
            tc.tile_pool(name="xp", bufs=3) as xpool,
            tc.tile_pool(name="wp", bufs=3) as wpool,
            tc.tile_pool(name="op", bufs=3) as opool,
            tc.tile_pool(name="ps", bufs=8, space="PSUM") as pspool,
        ):

            for b in range(B_LOC):
                # Per-k-chunk input DMAs so the first matmul of each batch
                # only waits on chunk 0; x rides the SP HWDGE ring, w the
                # ACT ring — they load in parallel, never queued behind
                # each other.
                xt = xpool.tile([128, KT, N_SEQ], f16, name=f"x_{b}", tag="x")
                wt = wpool.tile([128, KT, FOUT], f16, name=f"w_{b}", tag="w")
                for k in range(KT):
                    if b == 0 and k == 0:
                        # Halve the very first chunk loads: the opening
                        # k-outer pass only needs x cols 0-511 (m0-3) and
                        # w cols 0-511 (j0), so the first matmul starts a
                        # transfer-half earlier.
                        for h in range(2):
                            nc.sync.dma_start(
                                out=xt[:, 0, h * 512:(h + 1) * 512],
                                in_=xT[0, 0:KP, h * 512:(h + 1) * 512],
                            )
                            nc.scalar.dma_start(
                                out=wt[:, 0, h * 512:(h + 1) * 512],
                                in_=wT[0, 0:KP, h * 512:(h + 1) * 512],
                            )
                        continue
                    nc.sync.dma_start(
                        out=xt[:, k, :], in_=xT[b, k * KP:(k + 1) * KP, :]
                    )
                    nc.scalar.dma_start(
                        out=wt[:, k, :], in_=wT[b, k * KP:(k + 1) * KP, :]
                    )
                if b == 0:
                    # Batch 0 is gated by the input DMA stream: a k-inner
                    # group needs all 8 chunks within 1.7us while they
                    # arrive over ~10us.  Run the first half k-OUTER over 8
                    # concurrent PSUM banks so each chunk is consumed fully
                    # the moment it lands — no stream stalls.
                    MQ = MT // 4
                    psg = {}
                    for m in range(4):
                        for j in range(NT):
                            psg[m, j] = pspool.tile(
                                [128, NF], f32, name=f"ps_0h_{m}_{j}", tag="ps"
                            )
                    for k in range(KT):
                        for m in range(4):
                            for j in range(NT):
                                nc.tensor.matmul(
                                    psg[m, j],
                                    lhsT=xt[:, k, m * 128:(m + 1) * 128],
                                    rhs=wt[:, k, j * NF:(j + 1) * NF],
                                    start=(k == 0),
                                    stop=(k == KT - 1),
                                    skip_group_check=True,
                                )
                    for quarter in range(2):
                        stage = opool.tile(
                            [128, MQ, FOUT], f16, name=f"s_0_{quarter}", tag="s"
                        )
                        for mi in range(MQ):
                            m = quarter * MQ + mi
                            for j in range(NT):
                                nc.vector.tensor_copy(
                                    stage[:, mi, j * NF:(j + 1) * NF], psg[m, j]
                                )
                        nc.gpsimd.dma_start(
                            out=out[b, quarter * MQ * 128:(quarter + 1) * MQ * 128, :]
                            .rearrange("(m p) f -> p m f", p=128),
                            in_=stage,
                        )
                    quarters = range(2, 4)
                else:
                    quarters = range(4)
                for quarter in quarters:
                    MQ = MT // 4
                    stage = opool.tile([128, MQ, FOUT], f16, name=f"s_{b}_{quarter}", tag="s")
                    for mi in range(MQ):
                        m = quarter * MQ + mi
                        for j in range(NT):
                            ps = pspool.tile([128, NF], f32, name=f"ps_{b}_{m}_{j}", tag="ps")
                            for k in range(KT):
                                nc.tensor.matmul(
                                    ps,
                                    lhsT=xt[:, k, m * 128:(m + 1) * 128],
                                    rhs=wt[:, k, j * NF:(j + 1) * NF],
                                    start=(k == 0),
                                    stop=(k == KT - 1),
                                )
                            nc.vector.tensor_copy(
                                stage[:, mi, j * NF:(j + 1) * NF], ps
                            )
                    # Output DMAs ride the gpsimd SWDGE path so they never
                    # head-of-line block input prefetch; the final ones take
                    # the by-then-idle sync HWDGE ring (lower latency tail).
                    ring = (
                        nc.sync if (b == B_LOC - 1 and quarter >= 2)
                        else nc.gpsimd
                    )
                    ring.dma_start(
                        out=out[b, quarter * MQ * 128:(quarter + 1) * MQ * 128, :]
                        .rearrange("(m p) f -> p m f", p=128),
                        in_=stage,
                    )
    _split_excess_waits(nc)
    return nc


def _get_nc():
    global _NC_CACHE
    if _NC_CACHE is None:
        _NC_CACHE = _build_nc()
    return _NC_CACHE


def kernel(x, task_id, W):
    from concourse.bass_utils import run_bass_kernel_spmd

    x = np.asarray(x)
    task_id = np.asarray(task_id)
    W = np.asarray(W)

    # Host-side shard prep: fp16-cast + transpose x per batch, gather +
    # transpose the per-task weight (once per distinct task).
    xT = np.ascontiguousarray(x.astype(np.float16).transpose(0, 2, 1))  # [B, FIN, N]
    wT_by_task = {}
    for t in np.unique(task_id):
        t = int(t)
        wT_by_task[t] = np.ascontiguousarray(
            W[t].reshape(FOUT, FIN).astype(np.float16).T
        )

    in_maps = []
    for c in range(N_CORES):
        lo = c * B_LOC
        in_maps.append({
            "xT": xT[lo:lo + B_LOC],
            "wT": np.stack([wT_by_task[int(task_id[b])] for b in range(lo, lo + B_LOC)]),
        })

    nc = _get_nc()
    res = run_bass_kernel_spmd(nc, in_maps, core_ids=list(range(N_CORES)))
    out = np.concatenate(
        [res.results[c]["out"] for c in range(N_CORES)], axis=0
    ).astype(np.float32)
    return (out, task_id)
